# revision 1
# baseline (speedup 1.0000x reference)
"""Trainium2 Bass kernel for nn_FFT_features (conv1x1+BN+ReLU -> channel FFT ->
conv1x1+BN+ReLU -> channel iFFT magnitude -> conv1x1+BN+ReLU).

Key insight: the FFT/iFFT are over a 16-length channel axis, so they are tiny
dense linear maps.  The whole network collapses to a chain of small
channel-GEMMs + pointwise ops:

    y1  = relu(A1 @ x + c1)         A1 [16,3]   (BN1 folded into conv)
    y2  = relu(A2 @ y1 + c2)        A2 [32,16]  (= BN2*w_mid @ DFT, folded)
    zre = Gre @ y2 ; zim = Gim @ y2 Gre/Gim [16,32] (iFFT real/imag)
    mag = sqrt(zre^2 + zim^2)
    out = relu(A3 @ mag + c3)       A3 [64,16]  (BN3 folded)

Sharding: pure data parallel over 8 NeuronCores, each core takes 256 rows of
the flattened (B*H, W) pixel space (262144 pixels).

On-chip layout: channel GEMMs are stacked block-diagonally into the 128x128 PE
array (8x for stage 1, 4x for stage 2/3, 2x+4 row-tiles for stage 4) so the PE
streams full 128-wide outputs.  Pointwise work (bias+relu evictions, squares,
sqrt) is split between the Scalar (ACT) and Vector (DVE) engines.
"""

import os
import sys

for _p in ("/opt/trn_rl_repo", "/root/.axon_site", "/root/.axon_site/_ro/trn_rl_repo"):
    if os.path.isdir(_p) and _p not in sys.path:
        sys.path.append(_p)

import numpy as np
import ml_dtypes

import concourse.bass as bass
import concourse.bacc as bacc
import concourse.mybir as mybir
import concourse.tile as tile
from contextlib import ExitStack

F32 = mybir.dt.float32
BF16 = mybir.dt.bfloat16
F32R = mybir.dt.float32r

EPS = 1e-5
FCH = 16          # f = out_planes // 4
B, C, H, W = 4, 3, 512, 1024
OC = 64
N_CORES = 8
NPIX_CORE = (B * H * W) // N_CORES     # 262144
ROWS_CORE = (B * H) // N_CORES         # 256 rows of W pixels

# ---- kernel geometry ----
GSZ = 2048        # pixels per group within a load-tile
NG = 8            # groups stacked into the partition dim for stage 1
LT_PIX = GSZ * NG  # 16384 pixels per load-tile (one input DMA)
NQ = 4            # quanta (free-dim slices of 512) per load-tile
QN = 512          # matmul free dim

# variant: "bf16" | "f32r" | "f32"
VARIANT = os.environ.get("KERNEL_VARIANT", "bf16")
# how the stage-4 eviction halves are split between ACT and DVE
EV1_ENGINE = os.environ.get("KERNEL_EV1", "act")      # evict1 engine
SQ_FUSE = os.environ.get("KERNEL_SQ_FUSE", "0") == "1"  # use scalar_tensor_tensor pow fusion


def _fold_bn(w, g, b, m, v):
    s = g.astype(np.float64) / np.sqrt(v.astype(np.float64) + EPS)
    return s[:, None] * w.astype(np.float64), b.astype(np.float64) - m.astype(np.float64) * s


def make_host_weights(w_in, g1, b1, m1, v1, w_mid, g2, b2, m2, v2, w_out, g3, b3, m3, v3):
    """Fold BN + DFT/iDFT into 4 small matrices, then lay them out as the
    block-diagonal stacked lhsT tiles + per-partition bias vectors."""
    f = FCH
    A1, c1 = _fold_bn(w_in, g1, b1, m1, v1)            # [16,3]
    k = np.arange(f)
    F = np.exp(-2j * np.pi * np.outer(k, k) / f)
    Fmat = np.concatenate([F.real, F.imag], axis=0)     # [32,16]
    A2w, c2 = _fold_bn(w_mid, g2, b2, m2, v2)           # [32,32]
    A2 = A2w @ Fmat                                     # [32,16]
    co = np.cos(2 * np.pi * np.outer(k, k) / f) / f
    si = np.sin(2 * np.pi * np.outer(k, k) / f) / f
    G_re = np.concatenate([co, -si], axis=1)            # [16,32]
    G_im = np.concatenate([si, co], axis=1)             # [16,32]
    A3, c3 = _fold_bn(w_out, g3, b3, m3, v3)            # [64,16]

    lhsT1 = np.zeros((24, 128), np.float64)
    for g in range(8):
        # rhs partition 3g+c ; out partition 16g+o
        lhsT1[3 * g:3 * g + 3, 16 * g:16 * g + 16] = A1.T
    lhsT2 = np.zeros((128, 128), np.float64)
    for base in (0, 64):
        for gp in range(4):
            lhsT2[base + 16 * gp: base + 16 * gp + 16, 32 * gp:32 * gp + 32] = A2.T
    lhsT3 = np.zeros((128, 128), np.float64)
    for gp in range(4):
        lhsT3[32 * gp:32 * gp + 32, 16 * gp:16 * gp + 16] = G_re.T
        lhsT3[32 * gp:32 * gp + 32, 64 + 16 * gp:64 + 16 * gp + 16] = G_im.T
    lhsT4 = np.zeros((128, 128), np.float64)
    for t in range(4):
        for d in range(2):
            lhsT4[32 * t + 16 * d:32 * t + 16 * d + 16, 64 * d:64 * d + 64] = A3.T

    bias1 = np.tile(c1, 8).astype(np.float32).reshape(128, 1)
    bias2 = np.tile(c2, 4).astype(np.float32).reshape(128, 1)
    bias4 = np.tile(c3, 2).astype(np.float32).reshape(128, 1)
    return dict(lhsT1=lhsT1, lhsT2=lhsT2, lhsT3=lhsT3, lhsT4=lhsT4,
                bias1=bias1, bias2=bias2, bias4=bias4)


def build_nc(n_pix=NPIX_CORE, variant=VARIANT, ev1_engine=EV1_ENGINE, sq_fuse=SQ_FUSE):
    assert n_pix % LT_PIX == 0
    nlt = n_pix // LT_PIX

    if variant == "bf16":
        DT = BF16
    else:
        DT = F32

    def mmv(ap):
        # view an AP with the matmul dtype (f32r runs the PE at 1 col/cycle)
        if variant == "f32r":
            return ap.bitcast(F32R)
        return ap

    nc = bacc.Bacc("TRN2", target_bir_lowering=False, debug=False,
                   num_devices=N_CORES)
    img = nc.dram_tensor("img_slab", [3, n_pix], F32, kind="ExternalInput")
    wt1 = nc.dram_tensor("lhsT1", [24, 128], DT, kind="ExternalInput")
    wt2 = nc.dram_tensor("lhsT2", [128, 128], DT, kind="ExternalInput")
    wt3 = nc.dram_tensor("lhsT3", [128, 128], DT, kind="ExternalInput")
    wt4 = nc.dram_tensor("lhsT4", [128, 128], DT, kind="ExternalInput")
    bs1 = nc.dram_tensor("bias1", [128, 1], F32, kind="ExternalInput")
    bs2 = nc.dram_tensor("bias2", [128, 1], F32, kind="ExternalInput")
    bs4 = nc.dram_tensor("bias4", [128, 1], F32, kind="ExternalInput")
    out = nc.dram_tensor("out_slab", [64, n_pix], F32, kind="ExternalOutput")

    # DRAM views matching the on-chip partition layouts.  DMA matches source
    # and dest in flat AP-iteration order, so a [g,c,n] source view lines up
    # with a [(g c), n] SBUF tile, etc.
    in_view = img.rearrange("c (lt g n) -> lt g c n", lt=nlt, g=NG, n=GSZ)
    # out DMA per (load-tile, d): DRAM side [o, t, (tq j)] — 3 dims with an
    # 8KB contiguous inner run; SBUF side is a [64, 8192] contiguous slab.
    out_view = out.rearrange("o (lt t d n) -> lt d o t n",
                             lt=nlt, t=4, d=2, n=GSZ)

    Relu = mybir.ActivationFunctionType.Relu
    Sqrt = mybir.ActivationFunctionType.Sqrt
    Square = mybir.ActivationFunctionType.Square
    ADD = mybir.AluOpType.add
    MAX = mybir.AluOpType.max
    MULT = mybir.AluOpType.mult
    POW = mybir.AluOpType.pow

    with tile.TileContext(nc) as tc, ExitStack() as ctx:
        wpool = ctx.enter_context(tc.tile_pool(name="weights", bufs=1))
        lpool = ctx.enter_context(tc.tile_pool(name="load", bufs=3))
        y1pool = ctx.enter_context(tc.tile_pool(name="y1", bufs=3))
        y2pool = ctx.enter_context(tc.tile_pool(name="y2", bufs=3))
        sqpool = ctx.enter_context(tc.tile_pool(name="sq", bufs=2))
        magpool = ctx.enter_context(tc.tile_pool(name="mag", bufs=2))
        opool = ctx.enter_context(tc.tile_pool(name="ostage", bufs=2))
        p1pool = ctx.enter_context(tc.tile_pool(name="p1", bufs=2, space="PSUM"))
        p2pool = ctx.enter_context(tc.tile_pool(name="p2", bufs=1, space="PSUM"))
        p3repool = ctx.enter_context(tc.tile_pool(name="p3re", bufs=1, space="PSUM"))
        p3impool = ctx.enter_context(tc.tile_pool(name="p3im", bufs=1, space="PSUM"))
        p4pool = ctx.enter_context(tc.tile_pool(name="p4", bufs=1, space="PSUM"))

        lhsT1_sb = wpool.tile([24, 128], DT)
        nc.sync.dma_start(lhsT1_sb[:], wt1[:])
        lhsT2_sb = wpool.tile([128, 128], DT)
        nc.sync.dma_start(lhsT2_sb[:], wt2[:])
        lhsT3_sb = wpool.tile([128, 128], DT)
        nc.sync.dma_start(lhsT3_sb[:], wt3[:])
        lhsT4_sb = wpool.tile([128, 128], DT)
        nc.sync.dma_start(lhsT4_sb[:], wt4[:])
        bias1_sb = wpool.tile([128, 1], F32)
        nc.sync.dma_start(bias1_sb[:], bs1[:])
        bias2_sb = wpool.tile([128, 1], F32)
        nc.sync.dma_start(bias2_sb[:], bs2[:])
        bias4_sb = wpool.tile([128, 1], F32)
        nc.sync.dma_start(bias4_sb[:], bs4[:])

        def load_lt(i):
            Lt = lpool.tile([24, GSZ], DT, name="L", tag="L")
            if variant == "bf16":
                nc.gpsimd.dma_start(Lt[:], in_view[i])   # SWDGE casts f32->bf16
            else:
                nc.sync.dma_start(Lt[:], in_view[i])
            return Lt

        # ------------------------------------------------------------------
        # Software-pipelined emission: the per-quantum pointwise chain
        # alternates ACT and DVE; with in-order engine queues, emitting it
        # quantum-by-quantum serializes the two engines.  Instead emit with a
        # per-stage skew so each engine's stream interleaves independent ops
        # from different quanta.
        # ------------------------------------------------------------------
        Ltiles, y1s, y2s, Qlts, mags, Os = {}, {}, {}, {}, {}, {}
        nq_tot = nlt * NQ

        def phase_a(q):          # load prefetch + stage 1
            lt, tq = divmod(q, NQ)
            if tq == 0:
                if lt == 0:
                    for i in range(min(2, nlt)):
                        Ltiles[i] = load_lt(i)
                nxt = lt + 2
                if nxt < nlt:
                    Ltiles[nxt] = load_lt(nxt)
            L = Ltiles[q // NQ]
            P1 = p1pool.tile([128, QN], F32, name="P1", tag="p1")
            nc.tensor.matmul(P1[:], mmv(lhsT1_sb[:]), mmv(L[:, tq * QN:(tq + 1) * QN]))
            y1 = y1s[q] = y1pool.tile([128, QN], DT, name="y1", tag="y1")
            nc.scalar.activation(y1[:], P1[:], Relu, bias=bias1_sb[:])

        def phase_b(q):          # stage 2
            y1 = y1s.pop(q)
            P2 = p2pool.tile([128, 2 * QN], F32, name="P2", tag="p2")
            nc.tensor.matmul(P2[:, 0:QN], mmv(lhsT2_sb[0:64, :]), mmv(y1[0:64, :]))
            nc.tensor.matmul(P2[:, QN:2 * QN], mmv(lhsT2_sb[64:128, :]), mmv(y1[64:128, :]))
            y2 = y2s[q] = y2pool.tile([128, 2 * QN], DT, name="y2", tag="y2")
            nc.vector.tensor_scalar(y2[:], P2[:], bias2_sb[:], 0.0, ADD, MAX)

        def phase_c(q):          # stage 3 + squares
            lt, tq = divmod(q, NQ)
            y2 = y2s.pop(q)
            P3re = p3repool.tile([128, QN], F32, name="P3re", tag="p3re")
            P3im = p3impool.tile([128, QN], F32, name="P3im", tag="p3im")
            nc.tensor.matmul(P3re[0:64, :], mmv(lhsT3_sb[:, 0:64]), mmv(y2[:, 0:QN]))
            nc.tensor.matmul(P3re[64:128, :], mmv(lhsT3_sb[:, 0:64]), mmv(y2[:, QN:2 * QN]))
            nc.tensor.matmul(P3im[0:64, :], mmv(lhsT3_sb[:, 64:128]), mmv(y2[:, 0:QN]))
            nc.tensor.matmul(P3im[64:128, :], mmv(lhsT3_sb[:, 64:128]), mmv(y2[:, QN:2 * QN]))
            S1 = sqpool.tile([128, QN], F32, name="S1", tag="s1")
            nc.scalar.activation(S1[:], P3re[:], Square)
            S2 = sqpool.tile([128, QN], F32, name="S2", tag="s2")
            nc.scalar.activation(S2[:], P3im[:], Square)
            if tq == 0:
                Qlts[lt] = sqpool.tile([128, NQ * QN], F32, tag="q", name="Qlt")
            nc.vector.tensor_tensor(Qlts[lt][:, tq * QN:(tq + 1) * QN], S1[:], S2[:], ADD)

        def phase_d(q):          # batched sqrt once per load-tile
            lt, tq = divmod(q, NQ)
            if tq == NQ - 1:
                Q_lt = Qlts.pop(lt)
                mag = mags[lt] = magpool.tile([128, NQ * QN], DT, name="maglt", tag="mag")
                nc.scalar.activation(mag[:], Q_lt[:], Sqrt)

        def phase_e(q):          # stage 4 + output staging + out DMA
            lt, tq = divmod(q, NQ)
            if tq == 0:
                Os[lt] = opool.tile([128, 4 * NQ * QN], F32, name="Olt", tag="O")
            O = Os[lt]
            mag_lt = mags[lt]
            Ov = O[:].rearrange("p (t n) -> p t n", t=4)[:, :, tq * QN:(tq + 1) * QN]
            mg = mag_lt[:, tq * QN:(tq + 1) * QN]
            P4a = p4pool.tile([128, 2 * QN], F32, name="P4a", tag="p4")
            nc.tensor.matmul(P4a[:, 0:QN], mmv(lhsT4_sb[0:32, :]), mmv(mg[0:32, :]),
                             tile_position=(0, 0))
            nc.tensor.matmul(P4a[:, QN:2 * QN], mmv(lhsT4_sb[32:64, :]), mmv(mg[32:64, :]),
                             tile_position=(32, 0))
            nc.scalar.activation(Ov[:, 0:2, :], P4a[:], Relu, bias=bias4_sb[:])
            P4b = p4pool.tile([128, 2 * QN], F32, name="P4b", tag="p4")
            nc.tensor.matmul(P4b[:, 0:QN], mmv(lhsT4_sb[64:96, :]), mmv(mg[64:96, :]),
                             tile_position=(64, 0))
            nc.tensor.matmul(P4b[:, QN:2 * QN], mmv(lhsT4_sb[96:128, :]), mmv(mg[96:128, :]),
                             tile_position=(96, 0))
            nc.vector.tensor_scalar(Ov[:, 2:4, :], P4b[:], bias4_sb[:], 0.0, ADD, MAX)
            if tq == NQ - 1:
                mags.pop(lt)
                Os.pop(lt)
                for dd in range(2):
                    # 2MB SWDGE DMA; DRAM inner run is 8KB contiguous
                    nc.gpsimd.dma_start(out_view[lt, dd], O[64 * dd:64 * dd + 64, :])

        SKEW_B, SKEW_C, SKEW_D, SKEW_E = 1, 2, 3, 7
        for qq in range(nq_tot + SKEW_E):
            if qq < nq_tot:
                phase_a(qq)
            if 0 <= qq - SKEW_B < nq_tot:
                phase_b(qq - SKEW_B)
            if 0 <= qq - SKEW_C < nq_tot:
                phase_c(qq - SKEW_C)
            if 0 <= qq - SKEW_D < nq_tot:
                phase_d(qq - SKEW_D)
            if 0 <= qq - SKEW_E < nq_tot:
                phase_e(qq - SKEW_E)
    nc.compile()
    return nc


def host_pipeline(img_slab, hw):
    """Numpy model of exactly what the device computes (for sim verification)."""
    x = img_slab.astype(np.float64)                    # [3, n]
    A1 = hw["lhsT1"][0:3, 0:16].T
    y1 = np.maximum(A1 @ x + hw["bias1"][0:16], 0)
    A2 = hw["lhsT2"][0:16, 0:32].T
    y2 = np.maximum(A2 @ y1 + hw["bias2"][0:32], 0)
    Gre = hw["lhsT3"][0:32, 0:16].T
    Gim = hw["lhsT3"][0:32, 64:80].T
    zre = Gre @ y2
    zim = Gim @ y2
    mag = np.sqrt(zre * zre + zim * zim)
    A3 = hw["lhsT4"][0:16, 0:64].T
    y3 = np.maximum(A3 @ mag + hw["bias4"][0:64], 0)
    return y3.astype(np.float32)


_CACHE = {}


def _np_dt(variant):
    return ml_dtypes.bfloat16 if variant == "bf16" else np.float32


def kernel(img, w_in, g1, b1, m1, v1, w_mid, g2, b2, m2, v2, w_out, g3, b3, m3, v3,
           trace=False):
    from concourse.bass_utils import run_bass_kernel_spmd

    variant = VARIANT
    hw = make_host_weights(w_in, g1, b1, m1, v1, w_mid, g2, b2, m2, v2,
                           w_out, g3, b3, m3, v3)
    ndt = _np_dt(variant)
    weight_args = {
        "lhsT1": np.ascontiguousarray(hw["lhsT1"].astype(ndt)),
        "lhsT2": np.ascontiguousarray(hw["lhsT2"].astype(ndt)),
        "lhsT3": np.ascontiguousarray(hw["lhsT3"].astype(ndt)),
        "lhsT4": np.ascontiguousarray(hw["lhsT4"].astype(ndt)),
        "bias1": hw["bias1"], "bias2": hw["bias2"], "bias4": hw["bias4"],
    }

    key = variant
    if key not in _CACHE:
        _CACHE[key] = build_nc(variant=variant)
    nc = _CACHE[key]

    # core i handles rows [256i, 256(i+1)) of the flattened (B*H, W) space
    imgf = np.asarray(img, np.float32).reshape(B, 3, H * W)
    in_maps = []
    for i in range(N_CORES):
        b = (ROWS_CORE * i) // H
        h0 = (ROWS_CORE * i) % H
        slab = np.ascontiguousarray(
            imgf[b, :, h0 * W:(h0 + ROWS_CORE) * W])   # [3, NPIX_CORE]
        in_maps.append({"img_slab": slab, **weight_args})

    res = run_bass_kernel_spmd(nc, in_maps, list(range(N_CORES)), trace=trace)
    kernel.last_results = res

    outp = np.empty((B, OC, H, W), np.float32)
    for i in range(N_CORES):
        b = (ROWS_CORE * i) // H
        h0 = (ROWS_CORE * i) % H
        outp[b, :, h0:h0 + ROWS_CORE, :] = \
            res.results[i]["out_slab"].reshape(OC, ROWS_CORE, W)
    return outp


kernel.last_results = None



# revision 4
# speedup vs baseline: 1.1579x; 1.1579x over previous
"""Trainium2 Bass kernel for nn_FFT_features (conv1x1+BN+ReLU -> channel FFT ->
conv1x1+BN+ReLU -> channel iFFT magnitude -> conv1x1+BN+ReLU).

The FFT/iFFT are dense linear maps over a 16-length channel axis, so the whole
network collapses to a chain of small channel-GEMMs + pointwise ops:

    y1  = relu(A1 @ x + c1)         A1 [16,3]   (BN1 folded into conv)
    y2  = relu(A2 @ y1 + c2)        A2 [32,16]  (= BN2*w_mid @ DFT, folded)
    zre = Gre @ y2 ; zim = Gim @ y2 Gre/Gim [16,32] (iFFT real/imag)
    mag = sqrt(zre^2 + zim^2)
    out = relu(A3 @ mag + c3)       A3 [64,16]  (BN3 folded)

Sharding: pure data parallel over 8 NeuronCores, each core takes 256 rows of
the flattened (B*H, W) pixel space (262144 pixels).

v2 changes vs the f32 baseline (448us):
  - all HBM traffic is bf16: input is host-cast to bf16 (1.5MB/core) and the
    output slab is bf16 (32MB/core), upcast to f32 on the host.  This halves
    the dominant HBM write stream.
  - all DMA via HWDGE (nc.sync) instead of SWDGE: no cast-in-DMA needed any
    more, output staged in full 128-partition tiles with 16KB contiguous
    DRAM runs (the f32 baseline's 8KB-run SWDGE writes ran at ~14GB/s/engine,
    about half line rate).
  - input loaded once up-front into a persistent [24, 32768] SBUF tile in 4
    chunks (no per-load-tile input DMA at all).
  - mag-chain intermediates (squares, mag^2) in bf16: the S1+S2 add runs at
    DVE 2x packed rate.
  - engine split: ACT = y1-evict + sqrt + stage-4 evicts, DVE = y2-evict +
    squares + add.
"""

import os
import sys

for _p in ("/opt/trn_rl_repo", "/root/.axon_site", "/root/.axon_site/_ro/trn_rl_repo"):
    if os.path.isdir(_p) and _p not in sys.path:
        sys.path.append(_p)

import numpy as np
import ml_dtypes

import concourse.bass as bass
import concourse.bacc as bacc
import concourse.mybir as mybir
import concourse.tile as tile
from contextlib import ExitStack

F32 = mybir.dt.float32
BF16 = mybir.dt.bfloat16

EPS = 1e-5
FCH = 16          # f = out_planes // 4
B, C, H, W = 4, 3, 512, 1024
OC = 64
N_CORES = 8
NPIX_CORE = (B * H * W) // N_CORES     # 262144
ROWS_CORE = (B * H) // N_CORES         # 256 rows of W pixels

# ---- kernel geometry ----
GSZ = 2048        # pixels per group within a load-tile
NG = 8            # groups stacked into the partition dim for stage 1
LT_PIX = GSZ * NG  # 16384 pixels per load-tile
NLT = NPIX_CORE // LT_PIX  # 16
NQ = 4            # quanta (free-dim slices of 512) per load-tile
QN = 512          # matmul free dim
IN_COLS = NPIX_CORE // NG      # 32768 columns of the persistent input tile
OUT_COLS = NPIX_CORE * OC // 128  # 131072 columns of the bf16 output slab

# engine assignment knobs (tuned by measurement).  NOTE: squares must run on
# ACT (Square activation) — DVE tensor_tensor cannot read two PSUM operands.
SQ_ENGINE = os.environ.get("KERNEL_SQ", "act")      # squares: act only
E4A_ENGINE = os.environ.get("KERNEL_E4A", "act")    # stage-4 evict a
E4B_ENGINE = os.environ.get("KERNEL_E4B", "dve")    # stage-4 evict b
Y2_ENGINE = os.environ.get("KERNEL_Y2", "dve")      # y2 evict


def _fold_bn(w, g, b, m, v):
    s = g.astype(np.float64) / np.sqrt(v.astype(np.float64) + EPS)
    return s[:, None] * w.astype(np.float64), b.astype(np.float64) - m.astype(np.float64) * s


def make_host_weights(w_in, g1, b1, m1, v1, w_mid, g2, b2, m2, v2, w_out, g3, b3, m3, v3):
    """Fold BN + DFT/iDFT into 4 small matrices, then lay them out as the
    block-diagonal stacked lhsT tiles + per-partition bias vectors."""
    f = FCH
    A1, c1 = _fold_bn(w_in, g1, b1, m1, v1)            # [16,3]
    k = np.arange(f)
    F = np.exp(-2j * np.pi * np.outer(k, k) / f)
    Fmat = np.concatenate([F.real, F.imag], axis=0)     # [32,16]
    A2w, c2 = _fold_bn(w_mid, g2, b2, m2, v2)           # [32,32]
    A2 = A2w @ Fmat                                     # [32,16]
    co = np.cos(2 * np.pi * np.outer(k, k) / f) / f
    si = np.sin(2 * np.pi * np.outer(k, k) / f) / f
    G_re = np.concatenate([co, -si], axis=1)            # [16,32]
    G_im = np.concatenate([si, co], axis=1)             # [16,32]
    A3, c3 = _fold_bn(w_out, g3, b3, m3, v3)            # [64,16]

    lhsT1 = np.zeros((24, 128), np.float64)
    for g in range(8):
        # rhs partition 3g+c ; out partition 16g+o
        lhsT1[3 * g:3 * g + 3, 16 * g:16 * g + 16] = A1.T
    lhsT2 = np.zeros((128, 128), np.float64)
    for base in (0, 64):
        for gp in range(4):
            lhsT2[base + 16 * gp: base + 16 * gp + 16, 32 * gp:32 * gp + 32] = A2.T
    lhsT3 = np.zeros((128, 128), np.float64)
    for gp in range(4):
        lhsT3[32 * gp:32 * gp + 32, 16 * gp:16 * gp + 16] = G_re.T
        lhsT3[32 * gp:32 * gp + 32, 64 + 16 * gp:64 + 16 * gp + 16] = G_im.T
    lhsT4 = np.zeros((128, 128), np.float64)
    for t in range(4):
        for d in range(2):
            lhsT4[32 * t + 16 * d:32 * t + 16 * d + 16, 64 * d:64 * d + 64] = A3.T

    bias1 = np.tile(c1, 8).astype(np.float32).reshape(128, 1)
    bias2 = np.tile(c2, 4).astype(np.float32).reshape(128, 1)
    bias4 = np.tile(c3, 2).astype(np.float32).reshape(128, 1)
    return dict(lhsT1=lhsT1, lhsT2=lhsT2, lhsT3=lhsT3, lhsT4=lhsT4,
                bias1=bias1, bias2=bias2, bias4=bias4)


def build_nc(sq_engine=SQ_ENGINE, e4a_engine=E4A_ENGINE, e4b_engine=E4B_ENGINE,
             y2_engine=Y2_ENGINE):
    nc = bacc.Bacc("TRN2", target_bir_lowering=False, debug=False,
                   num_devices=N_CORES)
    img = nc.dram_tensor("img_slab", [24, IN_COLS], BF16, kind="ExternalInput")
    wt1 = nc.dram_tensor("lhsT1", [24, 128], BF16, kind="ExternalInput")
    wt2 = nc.dram_tensor("lhsT2", [128, 128], BF16, kind="ExternalInput")
    wt3 = nc.dram_tensor("lhsT3", [128, 128], BF16, kind="ExternalInput")
    wt4 = nc.dram_tensor("lhsT4", [128, 128], BF16, kind="ExternalInput")
    bs1 = nc.dram_tensor("bias1", [128, 1], F32, kind="ExternalInput")
    bs2 = nc.dram_tensor("bias2", [128, 1], F32, kind="ExternalInput")
    bs4 = nc.dram_tensor("bias4", [128, 1], F32, kind="ExternalInput")
    out = nc.dram_tensor("out_slab", [128, OUT_COLS], BF16, kind="ExternalOutput")

    # input loaded in 4 chunks of [24, 8192] (16KB contiguous runs per row)
    in_view = img.rearrange("p (k n) -> k p n", k=4)
    # one 2MB HWDGE write per load-tile, 16KB contiguous runs per row
    out_view = out.rearrange("p (lt n) -> lt p n", lt=NLT)

    Relu = mybir.ActivationFunctionType.Relu
    Sqrt = mybir.ActivationFunctionType.Sqrt
    Square = mybir.ActivationFunctionType.Square
    ADD = mybir.AluOpType.add
    MAX = mybir.AluOpType.max
    MULT = mybir.AluOpType.mult

    with tile.TileContext(nc) as tc, ExitStack() as ctx:
        wpool = ctx.enter_context(tc.tile_pool(name="weights", bufs=1))
        inpool = ctx.enter_context(tc.tile_pool(name="input", bufs=1))
        y1pool = ctx.enter_context(tc.tile_pool(name="y1", bufs=3))
        y2pool = ctx.enter_context(tc.tile_pool(name="y2", bufs=3))
        sqpool = ctx.enter_context(tc.tile_pool(name="sq", bufs=2))
        qpool = ctx.enter_context(tc.tile_pool(name="q", bufs=2))
        magpool = ctx.enter_context(tc.tile_pool(name="mag", bufs=2))
        opool = ctx.enter_context(tc.tile_pool(name="ostage", bufs=2))
        p1pool = ctx.enter_context(tc.tile_pool(name="p1", bufs=2, space="PSUM"))
        p2pool = ctx.enter_context(tc.tile_pool(name="p2", bufs=1, space="PSUM"))
        p3repool = ctx.enter_context(tc.tile_pool(name="p3re", bufs=1, space="PSUM"))
        p3impool = ctx.enter_context(tc.tile_pool(name="p3im", bufs=1, space="PSUM"))
        p4pool = ctx.enter_context(tc.tile_pool(name="p4", bufs=1, space="PSUM"))

        lhsT1_sb = wpool.tile([24, 128], BF16)
        nc.sync.dma_start(lhsT1_sb[:], wt1[:])
        lhsT2_sb = wpool.tile([128, 128], BF16)
        nc.sync.dma_start(lhsT2_sb[:], wt2[:])
        lhsT3_sb = wpool.tile([128, 128], BF16)
        nc.sync.dma_start(lhsT3_sb[:], wt3[:])
        lhsT4_sb = wpool.tile([128, 128], BF16)
        nc.sync.dma_start(lhsT4_sb[:], wt4[:])
        bias1_sb = wpool.tile([128, 1], F32)
        nc.sync.dma_start(bias1_sb[:], bs1[:])
        bias2_sb = wpool.tile([128, 1], F32)
        nc.sync.dma_start(bias2_sb[:], bs2[:])
        bias4_sb = wpool.tile([128, 1], F32)
        nc.sync.dma_start(bias4_sb[:], bs4[:])

        IN = inpool.tile([24, IN_COLS], BF16)

        def load_chunk(k):
            nc.sync.dma_start(IN[:, k * 8192:(k + 1) * 8192], in_view[k])

        def eng(name):
            return nc.scalar if name == "act" else nc.vector

        def evict_relu(engine, out_ap, in_ap, bias_ap):
            if engine == "act":
                nc.scalar.activation(out_ap, in_ap, Relu, bias=bias_ap)
            else:
                nc.vector.tensor_scalar(out_ap, in_ap, bias_ap, 0.0, ADD, MAX)

        # ------------------------------------------------------------------
        # Software-pipelined emission with per-stage skew, as in the
        # baseline: each engine's in-order stream interleaves independent
        # ops from different quanta.
        # ------------------------------------------------------------------
        y1s, y2s, Qlts, mags, Os = {}, {}, {}, {}, {}
        nq_tot = NLT * NQ

        def phase_a(q):          # input chunk prefetch + stage 1
            lt, tq = divmod(q, NQ)
            if tq == 0 and lt % 4 == 0:
                if lt == 0:
                    load_chunk(0)
                nxt = lt // 4 + 1
                if nxt < 4:
                    load_chunk(nxt)
            P1 = p1pool.tile([128, QN], F32, name="P1", tag="p1")
            col = lt * GSZ + tq * QN
            nc.tensor.matmul(P1[:], lhsT1_sb[:], IN[:, col:col + QN])
            y1 = y1s[q] = y1pool.tile([128, QN], BF16, name="y1", tag="y1")
            nc.scalar.activation(y1[:], P1[:], Relu, bias=bias1_sb[:])

        def phase_b(q):          # stage 2
            y1 = y1s.pop(q)
            P2 = p2pool.tile([128, 2 * QN], F32, name="P2", tag="p2")
            nc.tensor.matmul(P2[:, 0:QN], lhsT2_sb[0:64, :], y1[0:64, :])
            nc.tensor.matmul(P2[:, QN:2 * QN], lhsT2_sb[64:128, :], y1[64:128, :])
            y2 = y2s[q] = y2pool.tile([128, 2 * QN], BF16, name="y2", tag="y2")
            evict_relu(Y2_ENGINE, y2[:], P2[:], bias2_sb[:])

        def phase_c(q):          # stage 3 + squares + add
            lt, tq = divmod(q, NQ)
            y2 = y2s.pop(q)
            P3re = p3repool.tile([128, QN], F32, name="P3re", tag="p3re")
            P3im = p3impool.tile([128, QN], F32, name="P3im", tag="p3im")
            nc.tensor.matmul(P3re[0:64, :], lhsT3_sb[:, 0:64], y2[:, 0:QN])
            nc.tensor.matmul(P3re[64:128, :], lhsT3_sb[:, 0:64], y2[:, QN:2 * QN])
            nc.tensor.matmul(P3im[0:64, :], lhsT3_sb[:, 64:128], y2[:, 0:QN])
            nc.tensor.matmul(P3im[64:128, :], lhsT3_sb[:, 64:128], y2[:, QN:2 * QN])
            S1 = sqpool.tile([128, QN], BF16, name="S1", tag="s1")
            S2 = sqpool.tile([128, QN], BF16, name="S2", tag="s2")
            if sq_engine == "act":
                nc.scalar.activation(S1[:], P3re[:], Square)
                nc.scalar.activation(S2[:], P3im[:], Square)
            else:
                nc.vector.tensor_tensor(S1[:], P3re[:], P3re[:], MULT)
                nc.vector.tensor_tensor(S2[:], P3im[:], P3im[:], MULT)
            if tq == 0:
                Qlts[lt] = qpool.tile([128, NQ * QN], BF16, tag="q", name="Qlt")
            nc.vector.tensor_tensor(Qlts[lt][:, tq * QN:(tq + 1) * QN],
                                    S1[:], S2[:], ADD)

        def phase_d(q):          # batched sqrt once per load-tile
            lt, tq = divmod(q, NQ)
            if tq == NQ - 1:
                Q_lt = Qlts.pop(lt)
                mag = mags[lt] = magpool.tile([128, NQ * QN], BF16,
                                              name="maglt", tag="mag")
                nc.scalar.activation(mag[:], Q_lt[:], Sqrt)

        def phase_e(q):          # stage 4 + output staging + out DMA
            lt, tq = divmod(q, NQ)
            if tq == 0:
                Os[lt] = opool.tile([128, 4 * GSZ], BF16, name="Olt", tag="O")
            O = Os[lt]
            mag_lt = mags[lt]
            mg = mag_lt[:, tq * QN:(tq + 1) * QN]
            P4a = p4pool.tile([128, 2 * QN], F32, name="P4a", tag="p4")
            nc.tensor.matmul(P4a[:, 0:QN], lhsT4_sb[0:32, :], mg[0:32, :],
                             tile_position=(0, 0))
            nc.tensor.matmul(P4a[:, QN:2 * QN], lhsT4_sb[32:64, :], mg[32:64, :],
                             tile_position=(32, 0))
            evict_relu(e4a_engine, O[:, tq * 2048:tq * 2048 + 1024],
                       P4a[:], bias4_sb[:])
            P4b = p4pool.tile([128, 2 * QN], F32, name="P4b", tag="p4")
            nc.tensor.matmul(P4b[:, 0:QN], lhsT4_sb[64:96, :], mg[64:96, :],
                             tile_position=(64, 0))
            nc.tensor.matmul(P4b[:, QN:2 * QN], lhsT4_sb[96:128, :], mg[96:128, :],
                             tile_position=(96, 0))
            evict_relu(e4b_engine, O[:, tq * 2048 + 1024:tq * 2048 + 2048],
                       P4b[:], bias4_sb[:])
            if tq == NQ - 1:
                mags.pop(lt)
                Os.pop(lt)
                nc.sync.dma_start(out_view[lt], O[:])

        SKEW_B, SKEW_C, SKEW_D, SKEW_E = 1, 2, 3, 7
        for qq in range(nq_tot + SKEW_E):
            if qq < nq_tot:
                phase_a(qq)
            if 0 <= qq - SKEW_B < nq_tot:
                phase_b(qq - SKEW_B)
            if 0 <= qq - SKEW_C < nq_tot:
                phase_c(qq - SKEW_C)
            if 0 <= qq - SKEW_D < nq_tot:
                phase_d(qq - SKEW_D)
            if 0 <= qq - SKEW_E < nq_tot:
                phase_e(qq - SKEW_E)
    nc.compile()
    return nc


def host_pipeline(img_slab, hw):
    """Numpy model of exactly what the device computes (for verification).
    img_slab: [24, IN_COLS] (bf16)."""
    x = img_slab.astype(np.float64)
    A1 = hw["lhsT1"][0:3, 0:16].T
    A2 = hw["lhsT2"][0:16, 0:32].T
    Gre = hw["lhsT3"][0:32, 0:16].T
    Gim = hw["lhsT3"][0:32, 64:80].T
    A3 = hw["lhsT4"][0:16, 0:64].T
    outs = []
    for g in range(8):
        xg = x[3 * g:3 * g + 3]                         # [3, IN_COLS]
        y1 = np.maximum(A1 @ xg + hw["bias1"][16 * g:16 * g + 16], 0)
        y2 = np.maximum(A2 @ y1 + hw["bias2"][0:32], 0)
        mag = np.sqrt((Gre @ y2) ** 2 + (Gim @ y2) ** 2)
        outs.append(np.maximum(A3 @ mag + hw["bias4"][0:64], 0))
    return outs  # list of [64, IN_COLS] per group


_CACHE = {}


def kernel(img, w_in, g1, b1, m1, v1, w_mid, g2, b2, m2, v2, w_out, g3, b3, m3, v3,
           trace=False):
    from concourse.bass_utils import run_bass_kernel_spmd

    hw = make_host_weights(w_in, g1, b1, m1, v1, w_mid, g2, b2, m2, v2,
                           w_out, g3, b3, m3, v3)
    bf = ml_dtypes.bfloat16
    weight_args = {
        "lhsT1": np.ascontiguousarray(hw["lhsT1"].astype(bf)),
        "lhsT2": np.ascontiguousarray(hw["lhsT2"].astype(bf)),
        "lhsT3": np.ascontiguousarray(hw["lhsT3"].astype(bf)),
        "lhsT4": np.ascontiguousarray(hw["lhsT4"].astype(bf)),
        "bias1": hw["bias1"], "bias2": hw["bias2"], "bias4": hw["bias4"],
    }

    if "nc" not in _CACHE:
        _CACHE["nc"] = build_nc()
    nc = _CACHE["nc"]

    # core i handles rows [256i, 256(i+1)) of the flattened (B*H, W) space.
    # Per-core slab: [24, 32768] bf16 with row 3g+c = channel c of group g,
    # group g = contiguous 2048-pixel chunks: col = lt*2048 + j for pixel
    # lt*16384 + g*2048 + j.
    imgf = np.asarray(img, np.float32).reshape(B, 3, H * W)
    in_maps = []
    for i in range(N_CORES):
        b = (ROWS_CORE * i) // H
        h0 = (ROWS_CORE * i) % H
        slab = imgf[b, :, h0 * W:(h0 + ROWS_CORE) * W]      # [3, NPIX_CORE] f32
        # [3, 16, 8, 2048] -> [8, 3, 16, 2048] -> [24, 32768]
        arr = slab.reshape(3, NLT, NG, GSZ).transpose(2, 0, 1, 3) \
                  .reshape(24, IN_COLS).astype(bf)
        in_maps.append({"img_slab": np.ascontiguousarray(arr), **weight_args})

    res = run_bass_kernel_spmd(nc, in_maps, list(range(N_CORES)), trace=trace)
    kernel.last_results = res

    outp = np.empty((B, OC, H, W), np.float32)
    for i in range(N_CORES):
        b = (ROWS_CORE * i) // H
        h0 = (ROWS_CORE * i) % H
        slab = np.asarray(res.results[i]["out_slab"])        # [128, OUT_COLS] bf16
        # row 64d+o, col lt*8192 + q*2048 + t*512 + j
        #   -> pixel lt*16384 + g*2048 + q*512 + j with g = 4*(t//2)+2*(t%2)+d
        a = slab.reshape(2, 64, NLT, NQ, 2, 2, 512)          # [d,o,lt,q,h,tl,j]
        a = a.transpose(1, 2, 4, 5, 0, 3, 6)                 # [o,lt,h,tl,d,q,j]
        outp[b, :, h0:h0 + ROWS_CORE, :] = \
            a.reshape(OC, NPIX_CORE).astype(np.float32).reshape(OC, ROWS_CORE, W)
    return outp


kernel.last_results = None


# revision 12
# speedup vs baseline: 1.3050x; 1.1271x over previous
"""Trainium2 Bass kernel for nn_FFT_features (conv1x1+BN+ReLU -> channel FFT ->
conv1x1+BN+ReLU -> channel iFFT magnitude -> conv1x1+BN+ReLU).

The FFT/iFFT are dense linear maps over a 16-length channel axis, so the whole
network collapses to a chain of small channel-GEMMs + pointwise ops:

    y1  = relu(A1 @ x + c1)         A1 [16,3]   (BN1 folded into conv)
    y2  = relu(A2 @ y1 + c2)        A2 [32,16]  (= BN2*w_mid @ DFT, folded)
    zre = Gre @ y2 ; zim = Gim @ y2 Gre/Gim [16,32] (iFFT real/imag)
    mag = sqrt(zre^2 + zim^2)
    out = relu(A3 @ mag + c3)       A3 [64,16]  (BN3 folded)

Sharding: pure data parallel over 8 NeuronCores, each core takes 256 rows of
the flattened (B*H, W) pixel space (262144 pixels).

v2 changes vs the f32 baseline (448us):
  - all HBM traffic is bf16: input is host-cast to bf16 (1.5MB/core) and the
    output slab is bf16 (32MB/core), upcast to f32 on the host.  This halves
    the dominant HBM write stream.
  - all DMA via HWDGE (nc.sync) instead of SWDGE: no cast-in-DMA needed any
    more, output staged in full 128-partition tiles with 16KB contiguous
    DRAM runs (the f32 baseline's 8KB-run SWDGE writes ran at ~14GB/s/engine,
    about half line rate).
  - input loaded once up-front into a persistent [24, 32768] SBUF tile in 4
    chunks (no per-load-tile input DMA at all).
  - mag-chain intermediates (squares, mag^2) in bf16: the S1+S2 add runs at
    DVE 2x packed rate.
  - engine split: ACT = y1-evict + sqrt + stage-4 evicts, DVE = y2-evict +
    squares + add.
"""

import os
import sys

for _p in ("/opt/trn_rl_repo", "/root/.axon_site", "/root/.axon_site/_ro/trn_rl_repo"):
    if os.path.isdir(_p) and _p not in sys.path:
        sys.path.append(_p)

import numpy as np
import ml_dtypes

import concourse.bass as bass
import concourse.bacc as bacc
import concourse.mybir as mybir
import concourse.tile as tile
from contextlib import ExitStack

F32 = mybir.dt.float32
BF16 = mybir.dt.bfloat16

EPS = 1e-5
FCH = 16          # f = out_planes // 4
B, C, H, W = 4, 3, 512, 1024
OC = 64
N_CORES = 8
NPIX_CORE = (B * H * W) // N_CORES     # 262144
ROWS_CORE = (B * H) // N_CORES         # 256 rows of W pixels

# ---- kernel geometry ----
GSZ = 2048        # pixels per group within a load-tile
NG = 8            # groups stacked into the partition dim for stage 1
LT_PIX = GSZ * NG  # 16384 pixels per load-tile
NLT = NPIX_CORE // LT_PIX  # 16
NQ = 4            # quanta (free-dim slices of 512) per load-tile
QN = 512          # matmul free dim
IN_COLS = NPIX_CORE // NG      # 32768 columns of the persistent input tile
OUT_COLS = NPIX_CORE * OC // 128  # 131072 columns of the bf16 output slab

# engine assignment knobs (tuned by measurement).  NOTE: squares must run on
# ACT (Square activation) — DVE tensor_tensor cannot read two PSUM operands,
# and AluOpType.pow fails the ISA check on both DVE and GPSIMD.
E4_ENGINES = os.environ.get("KERNEL_E4", "aadd")    # stage-4 evict engines, one
                                                    # char per 512-col tile: a|d
Y1_ENGINE = os.environ.get("KERNEL_Y1", "act")      # y1 evict
Y2_ENGINE = os.environ.get("KERNEL_Y2", "dve")      # y2 evict
ADD_ENGINE = os.environ.get("KERNEL_ADD", "dve")    # S1+S2 add: dve | gps


def _fold_bn(w, g, b, m, v):
    s = g.astype(np.float64) / np.sqrt(v.astype(np.float64) + EPS)
    return s[:, None] * w.astype(np.float64), b.astype(np.float64) - m.astype(np.float64) * s


def make_host_weights(w_in, g1, b1, m1, v1, w_mid, g2, b2, m2, v2, w_out, g3, b3, m3, v3):
    """Fold BN + DFT/iDFT into 4 small matrices, then lay them out as the
    block-diagonal stacked lhsT tiles + per-partition bias vectors."""
    f = FCH
    A1, c1 = _fold_bn(w_in, g1, b1, m1, v1)            # [16,3]
    k = np.arange(f)
    F = np.exp(-2j * np.pi * np.outer(k, k) / f)
    Fmat = np.concatenate([F.real, F.imag], axis=0)     # [32,16]
    A2w, c2 = _fold_bn(w_mid, g2, b2, m2, v2)           # [32,32]
    A2 = A2w @ Fmat                                     # [32,16]
    co = np.cos(2 * np.pi * np.outer(k, k) / f) / f
    si = np.sin(2 * np.pi * np.outer(k, k) / f) / f
    G_re = np.concatenate([co, -si], axis=1)            # [16,32]
    G_im = np.concatenate([si, co], axis=1)             # [16,32]
    A3, c3 = _fold_bn(w_out, g3, b3, m3, v3)            # [64,16]

    lhsT1 = np.zeros((24, 128), np.float64)
    for g in range(8):
        # rhs partition 3g+c ; out partition 16g+o
        lhsT1[3 * g:3 * g + 3, 16 * g:16 * g + 16] = A1.T
    lhsT2 = np.zeros((128, 128), np.float64)
    for base in (0, 64):
        for gp in range(4):
            lhsT2[base + 16 * gp: base + 16 * gp + 16, 32 * gp:32 * gp + 32] = A2.T
    lhsT3 = np.zeros((128, 128), np.float64)
    for gp in range(4):
        lhsT3[32 * gp:32 * gp + 32, 16 * gp:16 * gp + 16] = G_re.T
        lhsT3[32 * gp:32 * gp + 32, 64 + 16 * gp:64 + 16 * gp + 16] = G_im.T
    lhsT4 = np.zeros((128, 128), np.float64)
    for t in range(4):
        for d in range(2):
            lhsT4[32 * t + 16 * d:32 * t + 16 * d + 16, 64 * d:64 * d + 64] = A3.T

    bias1 = np.tile(c1, 8).astype(np.float32).reshape(128, 1)
    bias2 = np.tile(c2, 4).astype(np.float32).reshape(128, 1)
    bias4 = np.tile(c3, 2).astype(np.float32).reshape(128, 1)
    return dict(lhsT1=lhsT1, lhsT2=lhsT2, lhsT3=lhsT3, lhsT4=lhsT4,
                bias1=bias1, bias2=bias2, bias4=bias4)


def build_nc(e4_engines=E4_ENGINES, y1_engine=Y1_ENGINE, y2_engine=Y2_ENGINE,
             add_engine=ADD_ENGINE):
    nc = bacc.Bacc("TRN2", target_bir_lowering=False, debug=False,
                   num_devices=N_CORES)
    img = nc.dram_tensor("img_slab", [24, IN_COLS], BF16, kind="ExternalInput")
    wt1 = nc.dram_tensor("lhsT1", [24, 128], BF16, kind="ExternalInput")
    wt2 = nc.dram_tensor("lhsT2", [128, 128], BF16, kind="ExternalInput")
    wt3 = nc.dram_tensor("lhsT3", [128, 128], BF16, kind="ExternalInput")
    wt4 = nc.dram_tensor("lhsT4", [128, 128], BF16, kind="ExternalInput")
    bs1 = nc.dram_tensor("bias1", [128, 1], F32, kind="ExternalInput")
    bs2 = nc.dram_tensor("bias2", [128, 1], F32, kind="ExternalInput")
    bs4 = nc.dram_tensor("bias4", [128, 1], F32, kind="ExternalInput")
    out = nc.dram_tensor("out_slab", [128, OUT_COLS], BF16, kind="ExternalOutput")

    # input loaded in 4 chunks of [24, 8192] (16KB contiguous runs per row)
    in_view = img.rearrange("p (k n) -> k p n", k=4)
    # one 2MB HWDGE write per load-tile, 16KB contiguous runs per row
    out_view = out.rearrange("p (lt n) -> lt p n", lt=NLT)

    Relu = mybir.ActivationFunctionType.Relu
    Sqrt = mybir.ActivationFunctionType.Sqrt
    Square = mybir.ActivationFunctionType.Square
    ADD = mybir.AluOpType.add
    MAX = mybir.AluOpType.max

    with tile.TileContext(nc) as tc, ExitStack() as ctx:
        wpool = ctx.enter_context(tc.tile_pool(name="weights", bufs=1))
        inpool = ctx.enter_context(tc.tile_pool(name="input", bufs=1))
        y1pool = ctx.enter_context(tc.tile_pool(name="y1", bufs=3))
        y2pool = ctx.enter_context(tc.tile_pool(name="y2", bufs=3))
        sqpool = ctx.enter_context(tc.tile_pool(name="sq", bufs=2))
        qpool = ctx.enter_context(tc.tile_pool(name="q", bufs=2))
        magpool = ctx.enter_context(tc.tile_pool(name="mag", bufs=2))
        opool = ctx.enter_context(tc.tile_pool(name="ostage", bufs=2))
        # PSUM bank budget (8 banks of [128, 512 f32]):
        #   p1 1 + p2 2 + p3 2 + p4 3 = 8.  p4 holds [128,512] tiles with
        #   bufs=3 so stage-4 matmuls stay 3 evictions ahead — otherwise the
        #   PE micro-stalls on the evicting engine every quantum, HAM
        #   re-throttles it to 1.2 GHz, and every matmul costs 2x.
        p1pool = ctx.enter_context(tc.tile_pool(name="p1", bufs=1, space="PSUM"))
        p2pool = ctx.enter_context(tc.tile_pool(name="p2", bufs=1, space="PSUM"))
        p3pool = ctx.enter_context(tc.tile_pool(name="p3", bufs=1, space="PSUM"))
        p4pool = ctx.enter_context(tc.tile_pool(name="p4", bufs=3, space="PSUM"))

        lhsT1_sb = wpool.tile([24, 128], BF16)
        nc.sync.dma_start(lhsT1_sb[:], wt1[:])
        lhsT2_sb = wpool.tile([128, 128], BF16)
        nc.sync.dma_start(lhsT2_sb[:], wt2[:])
        lhsT3_sb = wpool.tile([128, 128], BF16)
        nc.sync.dma_start(lhsT3_sb[:], wt3[:])
        lhsT4_sb = wpool.tile([128, 128], BF16)
        nc.sync.dma_start(lhsT4_sb[:], wt4[:])
        bias1_sb = wpool.tile([128, 1], F32)
        nc.sync.dma_start(bias1_sb[:], bs1[:])
        bias2_sb = wpool.tile([128, 1], F32)
        nc.sync.dma_start(bias2_sb[:], bs2[:])
        bias4_sb = wpool.tile([128, 1], F32)
        nc.sync.dma_start(bias4_sb[:], bs4[:])

        IN = inpool.tile([24, IN_COLS], BF16)

        def load_chunk(k):
            nc.sync.dma_start(IN[:, k * 8192:(k + 1) * 8192], in_view[k])

        def evict_relu(engine, out_ap, in_ap, bias_ap):
            if engine in ("act", "a"):
                nc.scalar.activation(out_ap, in_ap, Relu, bias=bias_ap)
            else:
                nc.vector.tensor_scalar(out_ap, in_ap, bias_ap, 0.0, ADD, MAX)

        # ------------------------------------------------------------------
        # Software-pipelined emission with per-stage skew, as in the
        # baseline: each engine's in-order stream interleaves independent
        # ops from different quanta.
        # ------------------------------------------------------------------
        y1s, y2s, Qlts, mags, Os = {}, {}, {}, {}, {}
        nq_tot = NLT * NQ

        def phase_a(q):          # input chunk prefetch + stage 1
            lt, tq = divmod(q, NQ)
            if tq == 0 and lt % 4 == 0:
                if lt == 0:
                    load_chunk(0)
                nxt = lt // 4 + 1
                if nxt < 4:
                    load_chunk(nxt)
            P1 = p1pool.tile([128, QN], F32, name="P1", tag="p1")
            col = lt * GSZ + tq * QN
            nc.tensor.matmul(P1[:], lhsT1_sb[:], IN[:, col:col + QN])
            y1 = y1s[q] = y1pool.tile([128, QN], BF16, name="y1", tag="y1")
            evict_relu(y1_engine, y1[:], P1[:], bias1_sb[:])

        def phase_b(q):          # stage 2
            y1 = y1s.pop(q)
            P2 = p2pool.tile([128, 2 * QN], F32, name="P2", tag="p2")
            nc.tensor.matmul(P2[:, 0:QN], lhsT2_sb[0:64, :], y1[0:64, :])
            nc.tensor.matmul(P2[:, QN:2 * QN], lhsT2_sb[64:128, :], y1[64:128, :])
            y2 = y2s[q] = y2pool.tile([128, 2 * QN], BF16, name="y2", tag="y2")
            evict_relu(y2_engine, y2[:], P2[:], bias2_sb[:])

        def phase_c(q):          # stage 3 + squares + add
            lt, tq = divmod(q, NQ)
            y2 = y2s.pop(q)
            # P3 [128, 1024]: cols 0:512 = re (both pixel halves stacked in
            # the partition dim), cols 512:1024 = im.  One Square covers both.
            P3 = p3pool.tile([128, 2 * QN], F32, name="P3", tag="p3")
            nc.tensor.matmul(P3[0:64, 0:QN], lhsT3_sb[:, 0:64], y2[:, 0:QN])
            nc.tensor.matmul(P3[64:128, 0:QN], lhsT3_sb[:, 0:64], y2[:, QN:2 * QN])
            nc.tensor.matmul(P3[0:64, QN:2 * QN], lhsT3_sb[:, 64:128], y2[:, 0:QN])
            nc.tensor.matmul(P3[64:128, QN:2 * QN], lhsT3_sb[:, 64:128],
                             y2[:, QN:2 * QN])
            S12 = sqpool.tile([128, 2 * QN], BF16, name="S12", tag="s12")
            nc.scalar.activation(S12[:], P3[:], Square)
            if tq == 0:
                Qlts[lt] = qpool.tile([128, NQ * QN], BF16, tag="q", name="Qlt")
            addeng = nc.gpsimd if add_engine == "gps" else nc.vector
            addeng.tensor_tensor(Qlts[lt][:, tq * QN:(tq + 1) * QN],
                                 S12[:, 0:QN], S12[:, QN:2 * QN], ADD)

        def phase_d(q):          # batched sqrt once per load-tile
            lt, tq = divmod(q, NQ)
            if tq == NQ - 1:
                Q_lt = Qlts.pop(lt)
                mag = mags[lt] = magpool.tile([128, NQ * QN], BF16,
                                              name="maglt", tag="mag")
                nc.scalar.activation(mag[:], Q_lt[:], Sqrt)

        def phase_e(q):          # stage 4 + output staging + out DMA
            lt, tq = divmod(q, NQ)
            if tq == 0:
                Os[lt] = opool.tile([128, 4 * GSZ], BF16, name="Olt", tag="O")
            O = Os[lt]
            mag_lt = mags[lt]
            mg = mag_lt[:, tq * QN:(tq + 1) * QN]
            for t in range(4):
                P4 = p4pool.tile([128, QN], F32, name="P4", tag="p4")
                nc.tensor.matmul(P4[:], lhsT4_sb[32 * t:32 * t + 32, :],
                                 mg[32 * t:32 * t + 32, :],
                                 tile_position=(32 * t, 0))
                evict_relu(e4_engines[t],
                           O[:, tq * 2048 + t * QN:tq * 2048 + (t + 1) * QN],
                           P4[:], bias4_sb[:])
            if tq == NQ - 1:
                mags.pop(lt)
                Os.pop(lt)
                nc.sync.dma_start(out_view[lt], O[:])

        SKEW_B, SKEW_C, SKEW_D, SKEW_E = 1, 2, 3, 7
        for qq in range(nq_tot + SKEW_E):
            if qq < nq_tot:
                phase_a(qq)
            if 0 <= qq - SKEW_B < nq_tot:
                phase_b(qq - SKEW_B)
            if 0 <= qq - SKEW_C < nq_tot:
                phase_c(qq - SKEW_C)
            if 0 <= qq - SKEW_D < nq_tot:
                phase_d(qq - SKEW_D)
            if 0 <= qq - SKEW_E < nq_tot:
                phase_e(qq - SKEW_E)
    nc.compile()
    return nc


def host_pipeline(img_slab, hw):
    """Numpy model of exactly what the device computes (for verification).
    img_slab: [24, IN_COLS] (bf16)."""
    x = img_slab.astype(np.float64)
    A1 = hw["lhsT1"][0:3, 0:16].T
    A2 = hw["lhsT2"][0:16, 0:32].T
    Gre = hw["lhsT3"][0:32, 0:16].T
    Gim = hw["lhsT3"][0:32, 64:80].T
    A3 = hw["lhsT4"][0:16, 0:64].T
    outs = []
    for g in range(8):
        xg = x[3 * g:3 * g + 3]                         # [3, IN_COLS]
        y1 = np.maximum(A1 @ xg + hw["bias1"][16 * g:16 * g + 16], 0)
        y2 = np.maximum(A2 @ y1 + hw["bias2"][0:32], 0)
        mag = np.sqrt((Gre @ y2) ** 2 + (Gim @ y2) ** 2)
        outs.append(np.maximum(A3 @ mag + hw["bias4"][0:64], 0))
    return outs  # list of [64, IN_COLS] per group


_CACHE = {}


def kernel(img, w_in, g1, b1, m1, v1, w_mid, g2, b2, m2, v2, w_out, g3, b3, m3, v3,
           trace=False):
    from concourse.bass_utils import run_bass_kernel_spmd

    hw = make_host_weights(w_in, g1, b1, m1, v1, w_mid, g2, b2, m2, v2,
                           w_out, g3, b3, m3, v3)
    bf = ml_dtypes.bfloat16
    weight_args = {
        "lhsT1": np.ascontiguousarray(hw["lhsT1"].astype(bf)),
        "lhsT2": np.ascontiguousarray(hw["lhsT2"].astype(bf)),
        "lhsT3": np.ascontiguousarray(hw["lhsT3"].astype(bf)),
        "lhsT4": np.ascontiguousarray(hw["lhsT4"].astype(bf)),
        "bias1": hw["bias1"], "bias2": hw["bias2"], "bias4": hw["bias4"],
    }

    if "nc" not in _CACHE:
        _CACHE["nc"] = build_nc()
    nc = _CACHE["nc"]

    # core i handles rows [256i, 256(i+1)) of the flattened (B*H, W) space.
    # Per-core slab: [24, 32768] bf16 with row 3g+c = channel c of group g,
    # group g = contiguous 2048-pixel chunks: col = lt*2048 + j for pixel
    # lt*16384 + g*2048 + j.
    imgf = np.asarray(img, np.float32).reshape(B, 3, H * W)
    in_maps = []
    for i in range(N_CORES):
        b = (ROWS_CORE * i) // H
        h0 = (ROWS_CORE * i) % H
        slab = imgf[b, :, h0 * W:(h0 + ROWS_CORE) * W]      # [3, NPIX_CORE] f32
        # [3, 16, 8, 2048] -> [8, 3, 16, 2048] -> [24, 32768]
        arr = slab.reshape(3, NLT, NG, GSZ).transpose(2, 0, 1, 3) \
                  .reshape(24, IN_COLS).astype(bf)
        in_maps.append({"img_slab": np.ascontiguousarray(arr), **weight_args})

    res = run_bass_kernel_spmd(nc, in_maps, list(range(N_CORES)), trace=trace)
    kernel.last_results = res

    outp = np.empty((B, OC, H, W), np.float32)
    for i in range(N_CORES):
        b = (ROWS_CORE * i) // H
        h0 = (ROWS_CORE * i) % H
        slab = np.asarray(res.results[i]["out_slab"])        # [128, OUT_COLS] bf16
        # row 64d+o, col lt*8192 + q*2048 + t*512 + j
        #   -> pixel lt*16384 + g*2048 + q*512 + j with g = 4*(t//2)+2*(t%2)+d
        a = slab.reshape(2, 64, NLT, NQ, 2, 2, 512)          # [d,o,lt,q,h,tl,j]
        a = a.transpose(1, 2, 4, 5, 0, 3, 6)                 # [o,lt,h,tl,d,q,j]
        outp[b, :, h0:h0 + ROWS_CORE, :] = \
            a.reshape(OC, NPIX_CORE).astype(np.float32).reshape(OC, ROWS_CORE, W)
    return outp


kernel.last_results = None
